# revision 1
# baseline (speedup 1.0000x reference)
"""Trainium2 Bass kernel for CLSAggregator: 6-layer dense transformer encoder
(ALiBi attention + SwiGLU MLP), B=4, S=1024, D=768, H=16, FF=3072.

Sharding: tokens split (batch, seq-half) -> 8 cores, 512 tokens each.
Per layer each core computes LN1/QKV/attention(local queries vs full keys of
its batch element)/Wo/LN2/SwiGLU for its token slab; K,V are exchanged within
core pairs via AllGather. Weights are replicated and streamed from HBM.

Self-contained: hardcodes all shapes; host side folds LN weights into the
projection weights and precomputes ALiBi distance/slope tensors.
"""
import math
import os

import numpy as np
import ml_dtypes

import concourse.bass as bass
import concourse.mybir as mybir
import concourse.tile as tile
from concourse import bacc
from concourse.bass_utils import run_bass_kernel_spmd
from concourse.masks import make_identity

F32 = mybir.dt.float32
F32R = mybir.dt.float32r
BF16 = mybir.dt.bfloat16
FP16 = mybir.dt.float16
AF = mybir.ActivationFunctionType
OP = mybir.AluOpType

L, H, D, FF = 6, 16, 768, 3072
B, NSEQ = 4, 1023
S = NSEQ + 1            # 1024
HD = D // H             # 48
EPS = 1e-5
NCORES = 8
T = S // 2              # 512 tokens per core
KT = D // 128           # 6 feature k-tiles
TT = T // 128           # 4 token tiles
FT = FF // 128          # 24 ff tiles
QK_PAD = H * 64         # 1024 padded q (or k) rows
VW = H * 49             # 784 v cols incl per-head ones-aug column

_DTMAP = {"bf16": BF16, "f32r": F32R, "fp16": FP16}
DT_A = _DTMAP[os.environ.get("KDT_A", "bf16")]   # attention operands / Wo / Wd / KV
DT_H = _DTMAP[os.environ.get("KDT_H", "f32r")]   # QKV & gate/up weights + acts

_NC_CACHE = {}


def head_spans(h):
    """Destination spans of head h's 48 rows inside 128-row feature tiles:
    list of (tile, dst_row, src_row_within_head, length)."""
    r = 48 * h
    g0, r0 = r // 128, r % 128
    if r0 + 48 <= 128:
        return [(g0, r0, 0, 48)]
    c = 128 - r0
    return [(g0, r0, 0, c), (g0 + 1, 0, c, 48 - c)]


def build_nc(use_bqk, use_bgu, l_run=L, bare=False, slopes=None):
    nc = bacc.Bacc("TRN2", target_bir_lowering=False, debug=False,
                   enable_asserts=True, num_devices=NCORES)

    # ---- I/O ----
    x0_d = nc.dram_tensor("x0", [T, D], F32, kind="ExternalInput")
    dist_d = nc.dram_tensor("dist", [S, T], FP16, kind="ExternalInput")
    if not os.environ.get("BIAS_DVE"):
        sid_d = nc.dram_tensor("sid", [128, H * 128], FP16, kind="ExternalInput")
    wqk_d = nc.dram_tensor("wqk", [L, D, 2 * QK_PAD], DT_H, kind="ExternalInput")
    wv_d = nc.dram_tensor("wv", [L, D, VW], DT_H, kind="ExternalInput")
    bv_d = nc.dram_tensor("bv", [L, 1, VW], DT_H, kind="ExternalInput")
    wo_d = nc.dram_tensor("wo", [L, QK_PAD, D], DT_A, kind="ExternalInput")
    wgu_d = nc.dram_tensor("wgu", [L, D, 2 * FF], DT_H, kind="ExternalInput")
    wd_d = nc.dram_tensor("wd", [L, FF, D], DT_A, kind="ExternalInput")
    if use_bqk:
        bqk_d = nc.dram_tensor("bqk", [L, 1, 2 * QK_PAD], DT_H, kind="ExternalInput")
    if use_bgu:
        bg_d = nc.dram_tensor("bg", [L, 1, FF], DT_H, kind="ExternalInput")
        bu_d = nc.dram_tensor("bu", [L, 1, FF], DT_H, kind="ExternalInput")
    finw_d = nc.dram_tensor("finw", [1, D], F32, kind="ExternalInput")
    finb_d = nc.dram_tensor("finb", [1, D], F32, kind="ExternalInput")
    y_d = nc.dram_tensor("y", [1, D], F32, kind="ExternalOutput")

    KELEM = QK_PAD * T
    VELEM = T * VW

    if bare:
        # overhead-measurement baseline: zero compute, same I/O signature
        with tile.TileContext(nc) as tc:
            with tc.tile_pool(name="pb", bufs=1) as pb:
                yt = pb.tile([1, D], F32, tag="fy", name="fy")
                nc.vector.memset(yt[:], 0.0)
                nc.sync.dma_start(y_d.ap(), yt[:])
        nc.compile()
        return nc

    with tile.TileContext(nc) as tc:
        with (
            tc.tile_pool(name="p1", bufs=1) as p1,
            tc.tile_pool(name="p2", bufs=2) as p2,
            tc.tile_pool(name="p3", bufs=3) as p3,
            tc.tile_pool(name="p4", bufs=4) as p4,
            tc.tile_pool(name="psmm", bufs=8, space="PSUM") as psmm,
            tc.tile_pool(name="dram", bufs=2, space="DRAM") as dram,
        ):
            # ---- persistent tiles ----
            ident = p1.tile([128, 128], F32, tag="ident", name="ident")
            make_identity(nc, ident[:])
            ones_f = p1.tile([1, 128], F32, tag="ones_f", name="ones_f")
            nc.vector.memset(ones_f[:], 1.0)
            ones_h = p1.tile([1, 128], DT_H, tag="ones_h", name="ones_h")    # K=1 lhsT for v bias
            nc.vector.tensor_copy(ones_h[:], ones_f[0:1, 0:128])
            if use_bqk or use_bgu:
                ones_row = p1.tile([1, T], DT_H, tag="ones_row", name="ones_row")
                nc.vector.memset(ones_row[:], 1.0) if False else None
                nc.scalar.copy(ones_row[0:1, 0:128], ones_f[:])
                nc.scalar.copy(ones_row[0:1, 128:256], ones_f[:])
                nc.scalar.copy(ones_row[0:1, 256:384], ones_f[:])
                nc.scalar.copy(ones_row[0:1, 384:512], ones_f[:])
            epst = p1.tile([128, 1], F32, tag="epst", name="epst")
            nc.vector.memset(epst[:], EPS)
            maskf = p1.tile([1, 64], F32, tag="maskf", name="maskf")
            nc.vector.memset(maskf[:], 0.0)
            nc.vector.memset(maskf[0:1, 0:48], 1.0)
            mask48 = p1.tile([1, 64], DT_A, tag="mask48", name="mask48")
            nc.vector.tensor_copy(mask48[:], maskf[:])

            # Pre-zero all PSUM banks so never-written pad regions read as
            # finite values (avoids NaN poisoning through 0-weight matmuls).
            zps = [psmm.tile([128, 512], F32, tag="mm", name="mm")
                   for _ in range(8)]
            for z in zps:
                nc.vector.memset(z[:], 0.0)

            x = [p1.tile([128, D], F32, tag=f"x{t}", name=f"x{t}") for t in range(TT)]
            for t in range(TT):
                nc.sync.dma_start(x[t][:], x0_d.ap()[t * 128:(t + 1) * 128, :])

            distT = [p1.tile([128, T], FP16, tag=f"dist{kt}", name=f"dist{kt}") for kt in range(8)]
            for kt in range(8):
                nc.sync.dma_start(distT[kt][:], dist_d.ap()[kt * 128:(kt + 1) * 128, :])
            if not os.environ.get("BIAS_DVE"):
                sid = p1.tile([128, H * 128], FP16, tag="sid", name="sid")
                nc.sync.dma_start(sid[:], sid_d.ap())

            def layernorm_to(src_tiles, dst_tiles):
                """LN over features (free dim of token-major src); transposed
                feature-major DT_H output into dst_tiles (KT x [128, T])."""
                for t in range(TT):
                    st = p2.tile([128, 12], F32, tag="bnst", name="bnst")
                    nc.vector.bn_stats(st[:, 0:6], src_tiles[t][:, 0:384])
                    nc.vector.bn_stats(st[:, 6:12], src_tiles[t][:, 384:768])
                    ag = p2.tile([128, 2], F32, tag="bnag", name="bnag")
                    nc.vector.bn_aggr(ag[:], st[:])
                    nmean = p2.tile([128, 1], F32, tag="nmean", name="nmean")
                    nc.scalar.mul(nmean[:], ag[:, 0:1], -1.0)
                    stdt = p2.tile([128, 1], F32, tag="stdt", name="stdt")
                    nc.scalar.activation(stdt[:], ag[:, 1:2], AF.Sqrt, bias=epst[:])
                    rstd = p2.tile([128, 1], F32, tag="rstd", name="rstd")
                    nc.vector.reciprocal(rstd[:], stdt[:])
                    hn = p2.tile([128, D], F32, tag="hnorm", name="hnorm")
                    nc.vector.tensor_scalar(hn[:], src_tiles[t][:], nmean[:], rstd[:],
                                            OP.add, OP.mult)
                    for d in range(KT):
                        pst = psmm.tile([128, 128], F32, tag="mm", name="mm")
                        nc.tensor.transpose(pst[:], hn[:, d * 128:(d + 1) * 128],
                                            ident[:])
                        nc.vector.tensor_copy(dst_tiles[d][:, t * 128:(t + 1) * 128],
                                              pst[:])

            for l in range(l_run):
                # ================= attention =================
                hT = [p1.tile([128, T], DT_H, tag=f"hT{k}", name=f"hT{k}") for k in range(KT)]
                layernorm_to(x, hT)

                kv_in = dram.tile([KELEM + VELEM], DT_A, tag="kv_in", name="kv_in")
                kv_out = dram.tile([2, KELEM + VELEM], DT_A, tag="kv_out", name="kv_out")
                kv_in_k = kv_in[0:KELEM].rearrange("(r c) -> r c", c=T)
                kv_in_v = kv_in[KELEM:KELEM + VELEM].rearrange("(r c) -> r c", c=VW)

                if use_bqk:
                    bqkt = p2.tile([1, 2 * QK_PAD], DT_H, tag="bqk_s", name="bqk_s")
                    nc.sync.dma_start(bqkt[:], bqk_d.ap()[l])

                # K projection (padded feature-major), staged to DRAM bounce
                for mp in range(4):
                    wt = p2.tile([128, KT, 256], DT_H, tag="wqk_s", name="wqk_s")
                    nc.sync.dma_start(
                        wt[:], wqk_d.ap()[l][:, QK_PAD + mp * 256:QK_PAD + (mp + 1) * 256]
                        .rearrange("(o p) n -> p o n", p=128))
                    for mm_ in range(2):
                        m = 2 * mp + mm_
                        ps = psmm.tile([128, T], F32, tag="mm", name="mm")
                        for k in range(KT):
                            nc.tensor.matmul(ps[:], wt[:, k, mm_ * 128:(mm_ + 1) * 128],
                                             hT[k][:], start=(k == 0),
                                             stop=(k == KT - 1 and not use_bqk))
                        if use_bqk:
                            nc.tensor.matmul(
                                ps[:], bqkt[0:1, QK_PAD + m * 128:QK_PAD + (m + 1) * 128],
                                ones_row[:], start=False, stop=True)
                        kst = p2.tile([128, T], DT_A, tag="k_stage", name="k_stage")
                        nc.vector.tensor_copy(kst[:], ps[:])
                        nc.sync.dma_start(kv_in_k[m * 128:(m + 1) * 128, :], kst[:])

                # V projection (token-major with ones-aug cols), staged
                bvt = p1.tile([1, VW], DT_H, tag="bv_s", name="bv_s")
                nc.sync.dma_start(bvt[:], bv_d.ap()[l])
                psv = [[psmm.tile([128, nlen], F32, tag="mm", name="mm")
                        for (n0, nlen) in ((0, 512), (512, VW - 512))]
                       for t in range(TT)]
                for k in range(KT):
                    wvt = p2.tile([128, VW], DT_H, tag="wv_s", name="wv_s")
                    nc.sync.dma_start(wvt[:],
                                      wv_d.ap()[l][k * 128:(k + 1) * 128, :])
                    for t in range(TT):
                        for ni, (n0, nlen) in enumerate(((0, 512), (512, VW - 512))):
                            nc.tensor.matmul(psv[t][ni][:],
                                             hT[k][:, t * 128:(t + 1) * 128],
                                             wvt[:, n0:n0 + nlen],
                                             start=(k == 0), stop=False)
                for t in range(TT):
                    vst = p2.tile([128, VW], DT_A, tag="v_stage", name="v_stage")
                    for ni, (n0, nlen) in enumerate(((0, 512), (512, VW - 512))):
                        nc.tensor.matmul(psv[t][ni][:], ones_h[:],
                                         bvt[0:1, n0:n0 + nlen], start=False, stop=True)
                        nc.vector.tensor_copy(vst[:, n0:n0 + nlen], psv[t][ni][:])
                    nc.sync.dma_start(kv_in_v[t * 128:(t + 1) * 128, :], vst[:])

                if os.environ.get("FAKE_AG"):
                    for c in range(2):
                        nc.sync.dma_start(
                            kv_out[c, :], kv_in[:])
                else:
                    nc.gpsimd.collective_compute(
                        "AllGather", OP.bypass,
                        replica_groups=[[0, 1], [2, 3], [4, 5], [6, 7]],
                        ins=[kv_in[:].opt()],
                        outs=[kv_out[:].opt()],
                    )

                # Q projection (padded feature-major), stays local
                qT = []
                for mp in range(4):
                    wt = p2.tile([128, KT, 256], DT_H, tag="wqk_s", name="wqk_s")
                    nc.sync.dma_start(
                        wt[:], wqk_d.ap()[l][:, mp * 256:(mp + 1) * 256]
                        .rearrange("(o p) n -> p o n", p=128))
                    for mm_ in range(2):
                        m = 2 * mp + mm_
                        ps = psmm.tile([128, T], F32, tag="mm", name="mm")
                        for k in range(KT):
                            nc.tensor.matmul(ps[:], wt[:, k, mm_ * 128:(mm_ + 1) * 128],
                                             hT[k][:], start=(k == 0),
                                             stop=(k == KT - 1 and not use_bqk))
                        if use_bqk:
                            nc.tensor.matmul(ps[:], bqkt[0:1, m * 128:(m + 1) * 128],
                                             ones_row[:], start=False, stop=True)
                        qt = p1.tile([128, T], DT_A, tag=f"qT{m}", name=f"qT{m}")
                        nc.vector.tensor_copy(qt[:], ps[:])
                        qT.append(qt)

                # assemble full-sequence K (feature-major) and V (token-major)
                kT_full = [p1.tile([128, S], DT_A, tag=f"kTf{r}", name=f"kTf{r}") for r in range(8)]
                for r in range(8):
                    for c in range(2):
                        nc.sync.dma_start(
                            kT_full[r][:, c * T:(c + 1) * T],
                            kv_out[c, r * 128 * T:(r + 1) * 128 * T]
                            .rearrange("(p f) -> p f", p=128))
                # padded to 800 cols so 64-wide per-head lhsT slices stay in range
                v_full = [p1.tile([128, VW + 16], DT_A, tag=f"vf{j}", name=f"vf{j}") for j in range(8)]
                for j in range(8):
                    c, jj = j // 4, j % 4
                    nc.vector.memset(v_full[j][:, VW:VW + 16], 0.0)
                    nc.sync.dma_start(
                        v_full[j][:, 0:VW],
                        kv_out[c, KELEM + jj * 128 * VW:KELEM + (jj + 1) * 128 * VW]
                        .rearrange("(p f) -> p f", p=128))

                # ---- per head-pair: scores + alibi bias, exp, AV (col-packed) ----
                # o_pad: 8 tiles [128, T]; pair j tile holds head 2j at rows
                # 0-47 (sum at 48) and head 2j+1 at rows 64-111 (sum at 112).
                o_pad = []
                sums_g = p1.tile([16, T], DT_A, tag="sums_g", name="sums_g")
                for j in range(8):
                    psav = psmm.tile([128, T], F32, tag="mm", name="mm")
                    for kt in range(8):
                        for hh, avb in ((2 * j, 0), (2 * j + 1, 64)):
                            ft, qb = hh // 2, (hh % 2) * 64
                            pss = psmm.tile([128, T], F32, tag="mm", name="mm")
                            pt = p4.tile([128, T], DT_A, tag="p", name="p")
                            if os.environ.get("BIAS_DVE"):
                                nc.tensor.matmul(
                                    pss[:],
                                    kT_full[ft][qb:qb + 64, kt * 128:(kt + 1) * 128],
                                    qT[ft][qb:qb + 64, :], start=True, stop=True)
                                ut = p4.tile([128, T], F32, tag="u", name="u")
                                nc.vector.tensor_tensor(ut[:], pss[:], distT[kt][:],
                                                        OP.subtract)
                                nc.scalar.activation(pt[:], ut[:], AF.Exp,
                                                     scale=float(slopes[hh]))
                            else:
                                nc.tensor.matmul(
                                    pss[:],
                                    kT_full[ft][qb:qb + 64, kt * 128:(kt + 1) * 128],
                                    qT[ft][qb:qb + 64, :], start=True, stop=False)
                                nc.tensor.matmul(
                                    pss[:], sid[:, hh * 128:(hh + 1) * 128],
                                    distT[kt][:], start=False, stop=True)
                                nc.scalar.activation(pt[:], pss[:], AF.Exp)
                            nc.tensor.matmul(
                                psav[avb:avb + 64, :],
                                v_full[kt][:, 49 * hh:49 * hh + 64], pt[:],
                                start=(kt == 0), stop=(kt == 7),
                                tile_position=(0, avb), skip_group_check=True)
                    oj = p1.tile([128, T], DT_A, tag=f"oall{j}", name=f"oall{j}")
                    nc.vector.tensor_copy(oj[:], psav[:])
                    # softmax denominators sit at rows 48 / 112 (v ones-aug)
                    nc.sync.dma_start(sums_g[2 * j:2 * j + 1, :], oj[48:49, :])
                    nc.sync.dma_start(sums_g[2 * j + 1:2 * j + 2, :], oj[112:113, :])
                    o_pad.append(oj)

                rec_f = p1.tile([16, T], F32, tag="rec_f", name="rec_f")
                nc.vector.reciprocal(rec_f[:], sums_g[:])
                rec_b = p1.tile([16, T], DT_A, tag="rec_b", name="rec_b")
                nc.scalar.copy(rec_b[:], rec_f[:])

                o_scaled = []
                for j in range(8):
                    ra = p4.tile([1, T], DT_A, tag="rec_row", name="rec_row")
                    nc.sync.dma_start(ra[:], rec_b[2 * j:2 * j + 1, :])
                    rb = p4.tile([1, T], DT_A, tag="rec_row", name="rec_row")
                    nc.sync.dma_start(rb[:], rec_b[2 * j + 1:2 * j + 2, :])
                    rep = psmm.tile([128, T], F32, tag="mm", name="mm")
                    nc.tensor.matmul(rep[0:64, :], mask48[:], ra[:],
                                     start=True, stop=True, tile_position=(0, 0))
                    nc.tensor.matmul(rep[64:128, :], mask48[:], rb[:],
                                     start=True, stop=True, tile_position=(0, 64))
                    rep_sb = p2.tile([128, T], DT_A, tag="rep_sb", name="rep_sb")
                    nc.scalar.copy(rep_sb[:], rep[:])
                    osj = p1.tile([128, T], DT_A, tag=f"opad{j}", name=f"opad{j}")
                    nc.vector.tensor_tensor(osj[:], o_pad[j][:], rep_sb[:], OP.mult)
                    o_scaled.append(osj)

                # ---- Wo (pad rows carry zero weights) + residual ----
                psw = [[psmm.tile([128, nlen], F32, tag="mm", name="mm")
                        for (n0, nlen) in ((0, 512), (512, 256))]
                       for t in range(TT)]
                for k in range(8):
                    wot = p3.tile([128, D], DT_A, tag="wo_s", name="wo_s")
                    nc.sync.dma_start(wot[:],
                                      wo_d.ap()[l][k * 128:(k + 1) * 128, :])
                    for t in range(TT):
                        for ni, (n0, nlen) in enumerate(((0, 512), (512, 256))):
                            nc.tensor.matmul(psw[t][ni][:],
                                             o_scaled[k][:, t * 128:(t + 1) * 128],
                                             wot[:, n0:n0 + nlen], start=(k == 0),
                                             stop=(k == 7))
                for t in range(TT):
                    for ni, (n0, nlen) in enumerate(((0, 512), (512, 256))):
                        nc.vector.tensor_tensor(x[t][:, n0:n0 + nlen],
                                                x[t][:, n0:n0 + nlen],
                                                psw[t][ni][:], OP.add)

                # ================= SwiGLU FFN =================
                h2T = [p1.tile([128, T], DT_H, tag=f"hT{k}", name=f"hT{k}") for k in range(KT)]
                layernorm_to(x, h2T)

                if use_bgu:
                    bgt = p2.tile([1, FF], DT_H, tag="bg_s", name="bg_s")
                    nc.sync.dma_start(bgt[:], bg_d.ap()[l])
                    but = p2.tile([1, FF], DT_H, tag="bu_s", name="bu_s")
                    nc.sync.dma_start(but[:], bu_d.ap()[l])
                h3 = []
                for f in range(FT):
                    wgu = p2.tile([128, KT, 256], DT_H, tag="wgu_s", name="wgu_s")
                    nc.sync.dma_start(
                        wgu[:], wgu_d.ap()[l][:, f * 256:(f + 1) * 256]
                        .rearrange("(o p) n -> p o n", p=128))
                    psg = psmm.tile([128, T], F32, tag="mm", name="mm")
                    for k in range(KT):
                        nc.tensor.matmul(psg[:], wgu[:, k, 0:128], h2T[k][:],
                                         start=(k == 0),
                                         stop=(k == KT - 1 and not use_bgu))
                    if use_bgu:
                        nc.tensor.matmul(psg[:], bgt[0:1, f * 128:(f + 1) * 128],
                                         ones_row[:], start=False, stop=True)
                    gsb = p2.tile([128, T], DT_A, tag="g_sb", name="g_sb")
                    if os.environ.get("SILU_DECOMP"):
                        sig = p2.tile([128, T], DT_A, tag="sig_sb", name="sig_sb")
                        nc.scalar.activation(sig[:], psg[:], AF.Sigmoid)
                        nc.vector.tensor_tensor(gsb[:], sig[:], psg[:], OP.mult)
                    else:
                        nc.scalar.activation(gsb[:], psg[:], AF.Silu)
                    psu = psmm.tile([128, T], F32, tag="mm", name="mm")
                    for k in range(KT):
                        nc.tensor.matmul(psu[:], wgu[:, k, 128:256], h2T[k][:],
                                         start=(k == 0),
                                         stop=(k == KT - 1 and not use_bgu))
                    if use_bgu:
                        nc.tensor.matmul(psu[:], but[0:1, f * 128:(f + 1) * 128],
                                         ones_row[:], start=False, stop=True)
                    h3f = p1.tile([128, T], DT_A, tag=f"h3_{f}", name=f"h3_{f}")
                    nc.vector.tensor_tensor(h3f[:], gsb[:], psu[:], OP.mult)
                    h3.append(h3f)

                psd = [[psmm.tile([128, nlen], F32, tag="mm", name="mm")
                        for (n0, nlen) in ((0, 512), (512, 256))] for t in range(TT)]
                for f in range(FT):
                    wdt = p3.tile([128, D], DT_A, tag="wd_s", name="wd_s")
                    nc.sync.dma_start(wdt[:], wd_d.ap()[l][f * 128:(f + 1) * 128, :])
                    for t in range(TT):
                        for ni, (n0, nlen) in enumerate(((0, 512), (512, 256))):
                            nc.tensor.matmul(psd[t][ni][:],
                                             h3[f][:, t * 128:(t + 1) * 128],
                                             wdt[:, n0:n0 + nlen],
                                             start=(f == 0), stop=(f == FT - 1))
                for t in range(TT):
                    for ni, (n0, nlen) in enumerate(((0, 512), (512, 256))):
                        nc.vector.tensor_tensor(x[t][:, n0:n0 + nlen],
                                                x[t][:, n0:n0 + nlen],
                                                psd[t][ni][:], OP.add)

            # ---- final layernorm of the CLS row (token 0) + affine ----
            finw = p1.tile([1, D], F32, tag="finw", name="finw")
            nc.sync.dma_start(finw[:], finw_d.ap())
            finb = p1.tile([1, D], F32, tag="finb", name="finb")
            nc.sync.dma_start(finb[:], finb_d.ap())

            x0r = x[0][0:1, :]
            fst = p2.tile([1, 12], F32, tag="fbnst", name="fbnst")
            nc.vector.bn_stats(fst[:, 0:6], x0r[:, 0:384])
            nc.vector.bn_stats(fst[:, 6:12], x0r[:, 384:768])
            fag = p2.tile([1, 2], F32, tag="fbnag", name="fbnag")
            nc.vector.bn_aggr(fag[:], fst[:])
            nmean = p2.tile([1, 1], F32, tag="fnmean", name="fnmean")
            nc.scalar.mul(nmean[:], fag[:, 0:1], -1.0)
            xc = p1.tile([1, D], F32, tag="fxc", name="fxc")
            nc.vector.tensor_scalar(xc[:], x0r, nmean[:], None, OP.add)
            stdt = p2.tile([1, 1], F32, tag="fstd", name="fstd")
            nc.scalar.activation(stdt[:], fag[:, 1:2], AF.Sqrt, bias=epst[0:1, :])
            rstd = p2.tile([1, 1], F32, tag="frstd", name="frstd")
            nc.vector.reciprocal(rstd[:], stdt[:])
            yt = p1.tile([1, D], F32, tag="fy", name="fy")
            nc.vector.tensor_scalar(yt[:], xc[:], rstd[:], None, OP.mult)
            nc.vector.tensor_tensor(yt[:], yt[:], finw[:], OP.mult)
            nc.vector.tensor_tensor(yt[:], yt[:], finb[:], OP.add)
            nc.sync.dma_start(y_d.ap(), yt[:])

    nc.compile()
    return nc


def prepare_inputs(cls_tokens, cls_token, log_slopes, Wqkv, Wo, Wg, Wu, Wd,
                   ln1_w, ln1_b, ln2_w, ln2_b, fin_w, fin_b):
    """Fold LN affine params into weights, pad heads, build per-core arrays."""
    f32 = np.float32
    bf16 = ml_dtypes.bfloat16
    scale = 1.0 / math.sqrt(HD)

    bias_dve = bool(os.environ.get("BIAS_DVE"))
    slopes_np = np.exp(np.asarray(log_slopes, np.float64))
    wqk = np.zeros((L, D, 2 * QK_PAD), f32)
    bqk = np.zeros((L, 1, 2 * QK_PAD), f32)
    wv = np.zeros((L, D, VW), f32)
    bv = np.zeros((L, 1, VW), f32)
    wo = np.zeros((L, QK_PAD, D), f32)
    wgu = np.zeros((L, D, 2 * FF), f32)
    bg = np.zeros((L, 1, FF), f32)
    bu = np.zeros((L, 1, FF), f32)
    wd = np.zeros((L, FF, D), f32)

    for l in range(L):
        W1 = (np.asarray(Wqkv[l], np.float64) *
              np.asarray(ln1_w[l], np.float64)[None, :])
        b1 = np.asarray(Wqkv[l], np.float64) @ np.asarray(ln1_b[l], np.float64)
        for h in range(H):
            qs = slice(48 * h, 48 * h + 48)
            qscale = scale / slopes_np[h] if bias_dve else scale
            wqk[l, :, 64 * h:64 * h + 48] = (W1[qs].T * qscale)
            bqk[l, 0, 64 * h:64 * h + 48] = b1[qs] * qscale
            ks = slice(D + 48 * h, D + 48 * h + 48)
            wqk[l, :, QK_PAD + 64 * h:QK_PAD + 64 * h + 48] = W1[ks].T
            bqk[l, 0, QK_PAD + 64 * h:QK_PAD + 64 * h + 48] = b1[ks]
            vs = slice(2 * D + 48 * h, 2 * D + 48 * h + 48)
            wv[l, :, 49 * h:49 * h + 48] = W1[vs].T
            bv[l, 0, 49 * h:49 * h + 48] = b1[vs]
            bv[l, 0, 49 * h + 48] = 1.0
        woT = np.asarray(Wo[l], f32).T
        for h in range(H):
            wo[l, 64 * h:64 * h + 48, :] = woT[48 * h:48 * h + 48, :]
        W2g = (np.asarray(Wg[l], np.float64) *
               np.asarray(ln2_w[l], np.float64)[None, :])
        W2u = (np.asarray(Wu[l], np.float64) *
               np.asarray(ln2_w[l], np.float64)[None, :])
        wgu_l = wgu[l].reshape(D, FT, 2, 128)
        wgu_l[:, :, 0, :] = W2g.T.reshape(D, FT, 128)
        wgu_l[:, :, 1, :] = W2u.T.reshape(D, FT, 128)
        bg[l, 0] = np.asarray(Wg[l], np.float64) @ np.asarray(ln2_b[l], np.float64)
        bu[l, 0] = np.asarray(Wu[l], np.float64) @ np.asarray(ln2_b[l], np.float64)
        wd[l] = np.asarray(Wd[l], f32).T

    use_bqk = bool(np.any(bqk != 0))
    use_bgu = bool(np.any(bg != 0) or np.any(bu != 0))

    sid = np.zeros((128, H * 128), np.float16)
    for h in range(H):
        sid[:, h * 128:(h + 1) * 128] = -slopes_np[h] * np.eye(128)

    x_full = np.concatenate(
        [np.broadcast_to(np.asarray(cls_token, f32), (B, 1, D)),
         np.asarray(cls_tokens, f32)], axis=1)  # (B, S, D)

    np_a = mybir.dt.np(DT_A)
    np_h = mybir.dt.np(DT_H)
    common = {
        "wqk": wqk.astype(np_h), "wv": wv.astype(np_h), "bv": bv.astype(np_h),
        "wo": wo.astype(np_a), "wgu": wgu.astype(np_h),
        "wd": wd.astype(np_a),
        "finw": np.asarray(fin_w, f32).reshape(1, D),
        "finb": np.asarray(fin_b, f32).reshape(1, D),
    }
    if not bias_dve:
        common["sid"] = sid
    if use_bqk:
        common["bqk"] = bqk.astype(np_h)
    if use_bgu:
        common["bg"] = bg.astype(np_h)
        common["bu"] = bu.astype(np_h)

    ks = np.arange(S, dtype=np.float64)
    in_maps = []
    for c in range(NCORES):
        b, half = c // 2, c % 2
        q0 = T * half
        dist = np.abs((q0 + np.arange(T, dtype=np.float64))[None, :] - ks[:, None])
        m = dict(common)
        m["x0"] = np.ascontiguousarray(x_full[b, q0:q0 + T])
        m["dist"] = dist.astype(np.float16)
        in_maps.append(m)
    return in_maps, use_bqk, use_bgu


def kernel(**inputs):
    in_maps, use_bqk, use_bgu = prepare_inputs(**inputs)
    slopes = np.exp(np.asarray(inputs["log_slopes"], np.float64))
    key = (use_bqk, use_bgu, tuple(np.round(slopes, 10)))
    if key not in _NC_CACHE:
        _NC_CACHE[key] = build_nc(use_bqk, use_bgu, slopes=slopes)
    nc = _NC_CACHE[key]
    res = run_bass_kernel_spmd(nc, in_maps, core_ids=list(range(NCORES)))
    out = np.stack([res.results[2 * b]["y"][0] for b in range(B)])
    return out.astype(np.float32)



# revision 17
# speedup vs baseline: 4.3343x; 4.3343x over previous
"""Trainium2 Bass kernel for CLSAggregator: 6-layer dense transformer encoder
(ALiBi attention + SwiGLU MLP), B=4, S=1024, D=768, H=16, FF=3072.

Sharding: tokens split (batch, seq-half) -> 8 cores, 512 tokens each.
Per layer each core computes LN1/QKV/attention(local queries vs full keys of
its batch element)/Wo/LN2/SwiGLU for its token slab; K,V are exchanged within
core pairs via AllGather. Weights are replicated and streamed from HBM.

v2 layout notes:
- fp16 operands throughout (PE runs fp16 at 1 cycle/row, halves DMA vs f32r).
- ALiBi bias applied off the PE: scores PSUM minus dist (DVE/Pool), then
  exp with per-head slope as the activation scale. Q is pre-scaled by
  scale/slope_h on the host.
- K is projected and AllGathered dense (768 rows); scores slice 48-row heads
  directly (matmul cost only depends on the moving dim).
- AV is computed in [q, vcol] orientation (N=49 incl. denominator column),
  so softmax normalization is a per-partition tensor_scalar.
- FFN streams gate/up/down per 128-wide f-tile (h3 freed immediately).
- Last layer computes only the first 128-token block of queries (CLS lives
  at token 0 of even cores).

Self-contained: hardcodes all shapes; host side folds LN weights into the
projection weights and precomputes ALiBi distance tensors.
"""
import math
import os

import numpy as np
import ml_dtypes

import concourse.bass as bass
import concourse.mybir as mybir
import concourse.tile as tile
from concourse import bacc
from concourse.bass_utils import run_bass_kernel_spmd
from concourse.masks import make_identity

F32 = mybir.dt.float32
F32R = mybir.dt.float32r
BF16 = mybir.dt.bfloat16
FP16 = mybir.dt.float16
AF = mybir.ActivationFunctionType
OP = mybir.AluOpType

L, H, D, FF = 6, 16, 768, 3072
B, NSEQ = 4, 1023
S = NSEQ + 1            # 1024
HD = D // H             # 48
EPS = 1e-5
NCORES = 8
T = S // 2              # 512 tokens per core
KT = D // 128           # 6 feature k-tiles
TT = T // 128           # 4 token tiles
FT = FF // 128          # 24 ff tiles
QK_PAD = H * 64         # 1024 padded q rows
VW = H * 49             # 784 v cols incl per-head ones-aug column

_DTMAP = {"bf16": BF16, "f32r": F32R, "fp16": FP16}
DT_A = _DTMAP[os.environ.get("KDT_A", "fp16")]   # attention operands / Wo / Wd
DT_H = _DTMAP[os.environ.get("KDT_H", "fp16")]   # QKV & gate/up weights + acts

_NC_CACHE = {}


def build_nc(use_bqk, use_bgu, l_run=L, bare=False, slopes=None):
    nc = bacc.Bacc("TRN2", target_bir_lowering=False, debug=False,
                   enable_asserts=True, num_devices=NCORES)

    # ---- I/O ----
    x0_d = nc.dram_tensor("x0", [T, D], F32, kind="ExternalInput")
    dist_d = nc.dram_tensor("dist", [S, T], FP16, kind="ExternalInput")
    wq_d = nc.dram_tensor("wq", [L, D, QK_PAD], DT_H, kind="ExternalInput")
    wk_d = nc.dram_tensor("wk", [L, D, D], DT_H, kind="ExternalInput")
    wv_d = nc.dram_tensor("wv", [L, D, VW], DT_H, kind="ExternalInput")
    bv_d = nc.dram_tensor("bv", [L, 1, VW], DT_H, kind="ExternalInput")
    wo_d = nc.dram_tensor("wo", [L, D, D], DT_A, kind="ExternalInput")
    wgu_d = nc.dram_tensor("wgu", [L, D, 2 * FF], DT_H, kind="ExternalInput")
    wd_d = nc.dram_tensor("wd", [L, FF, D], DT_A, kind="ExternalInput")
    if use_bqk:
        bqk_d = nc.dram_tensor("bqk", [L, 1, QK_PAD + D], DT_H, kind="ExternalInput")
    if use_bgu:
        bg_d = nc.dram_tensor("bg", [L, 1, FF], DT_H, kind="ExternalInput")
        bu_d = nc.dram_tensor("bu", [L, 1, FF], DT_H, kind="ExternalInput")
    finw_d = nc.dram_tensor("finw", [1, D], F32, kind="ExternalInput")
    finb_d = nc.dram_tensor("finb", [1, D], F32, kind="ExternalInput")
    y_d = nc.dram_tensor("y", [1, D], F32, kind="ExternalOutput")

    KELEM = D * T
    VELEM = T * VW

    if bare:
        # overhead-measurement baseline: zero compute, same I/O signature
        with tile.TileContext(nc) as tc:
            with tc.tile_pool(name="pb", bufs=1) as pb:
                yt = pb.tile([1, D], F32, tag="fy", name="fy")
                nc.vector.memset(yt[:], 0.0)
                nc.sync.dma_start(y_d.ap(), yt[:])
        nc.compile()
        return nc

    with tile.TileContext(nc) as tc:
        with (
            tc.tile_pool(name="p1", bufs=1) as p1,
            tc.tile_pool(name="p2", bufs=2) as p2,
            tc.tile_pool(name="p3", bufs=3) as p3,
            tc.tile_pool(name="p4", bufs=4) as p4,
            tc.tile_pool(name="psmm", bufs=8, space="PSUM") as psmm,
            tc.tile_pool(name="dram", bufs=2, space="DRAM") as dram,
        ):
            # ---- persistent tiles ----
            ident = p1.tile([128, 128], F32, tag="ident", name="ident")
            make_identity(nc, ident[:])
            ident_a = p1.tile([128, 128], DT_A, tag="ident_a", name="ident_a")
            nc.vector.tensor_copy(ident_a[:], ident[:])
            negid_f = p1.tile([128, 128], F32, tag="negid_f", name="negid_f")
            nc.scalar.mul(negid_f[:], ident[:], -1.0)
            negid = p1.tile([128, 128], FP16, tag="negid", name="negid")
            nc.vector.tensor_copy(negid[:], negid_f[:])
            ident_h = p1.tile([128, 128], DT_H, tag="ident_h", name="ident_h")
            nc.vector.tensor_copy(ident_h[:], ident[:])
            ones_f = p1.tile([1, 128], F32, tag="ones_f", name="ones_f")
            nc.vector.memset(ones_f[:], 1.0)
            ones_h = p1.tile([1, 128], DT_H, tag="ones_h", name="ones_h")
            nc.vector.tensor_copy(ones_h[:], ones_f[0:1, 0:128])
            if use_bqk or use_bgu:
                ones_row = p1.tile([1, T], DT_H, tag="ones_row", name="ones_row")
                for c4 in range(4):
                    nc.scalar.copy(ones_row[0:1, c4 * 128:(c4 + 1) * 128], ones_f[:])
            epst = p1.tile([128, 1], F32, tag="epst", name="epst")
            nc.vector.memset(epst[:], EPS)

            # Pre-zero all PSUM banks once (pool-reused later).
            zps = [psmm.tile([128, 512], F32, tag="mm", name="mm")
                   for _ in range(8)]
            for z in zps:
                nc.vector.memset(z[:], 0.0)

            x = [p1.tile([128, D], F32, tag=f"x{t}", name=f"x{t}") for t in range(TT)]
            for t in range(TT):
                nc.sync.dma_start(x[t][:], x0_d.ap()[t * 128:(t + 1) * 128, :])

            distT = [p1.tile([128, T], FP16, tag=f"dist{kt}", name=f"dist{kt}") for kt in range(8)]
            for kt in range(8):
                nc.sync.dma_start(distT[kt][:], dist_d.ap()[kt * 128:(kt + 1) * 128, :])

            def layernorm_to(src_tiles, dst_tiles, tlist):
                """LN over features (free dim of token-major src); transposed
                feature-major DT_H output into dst_tiles token-cols t*128.."""
                for t in tlist:
                    st = p2.tile([128, 12], F32, tag="bnst", name="bnst")
                    nc.vector.bn_stats(st[:, 0:6], src_tiles[t][:, 0:384])
                    nc.vector.bn_stats(st[:, 6:12], src_tiles[t][:, 384:768])
                    ag = p2.tile([128, 2], F32, tag="bnag", name="bnag")
                    nc.vector.bn_aggr(ag[:], st[:])
                    nmean = p2.tile([128, 1], F32, tag="nmean", name="nmean")
                    nc.scalar.mul(nmean[:], ag[:, 0:1], -1.0)
                    stdt = p2.tile([128, 1], F32, tag="stdt", name="stdt")
                    nc.scalar.activation(stdt[:], ag[:, 1:2], AF.Sqrt, bias=epst[:])
                    rstd = p2.tile([128, 1], F32, tag="rstd", name="rstd")
                    nc.vector.reciprocal(rstd[:], stdt[:])
                    hn = p2.tile([128, D], DT_H, tag="hnorm", name="hnorm")
                    nc.vector.tensor_scalar(hn[:], src_tiles[t][:], nmean[:], rstd[:],
                                            OP.add, OP.mult)
                    for d in range(KT):
                        pst = psmm.tile([128, 128], DT_H, tag="mm", name="mm")
                        nc.tensor.transpose(pst[:], hn[:, d * 128:(d + 1) * 128],
                                            ident_h[:])
                        nc.vector.tensor_copy(dst_tiles[d][:, t * 128:(t + 1) * 128],
                                              pst[:])

            KC = 256                      # tokens per kv pipeline chunk
            KELEMC = D * KC
            VELEMC = KC * VW

            def kv_pipeline(l, hT_l, chunk):
                """Project K (dense) + V for one 256-token chunk, stage to the
                DRAM bounce buffer, and kick the pair AllGather."""
                c0 = chunk * KC
                kv_in = dram.tile([KELEMC + VELEMC], DT_A,
                                  tag=f"kv_in{chunk}", name=f"kv_in{chunk}")
                kv_out = dram.tile([2, KELEMC + VELEMC], DT_A,
                                   tag=f"kv_out{chunk}", name=f"kv_out{chunk}")
                kv_in_k = kv_in[0:KELEMC].rearrange("(r c) -> r c", c=KC)
                kv_in_v = kv_in[KELEMC:KELEMC + VELEMC].rearrange("(r c) -> r c", c=VW)

                for m in range(KT):
                    wt = p2.tile([128, KT, 128], DT_H, tag="wk_s", name="wk_s")
                    nc.sync.dma_start(
                        wt[:], wk_d.ap()[l][:, m * 128:(m + 1) * 128]
                        .rearrange("(o p) n -> p o n", p=128))
                    ps = psmm.tile([128, KC], F32, tag="mm", name="mm")
                    for k in range(KT):
                        nc.tensor.matmul(ps[:], wt[:, k], hT_l[k][:, c0:c0 + KC],
                                         start=(k == 0),
                                         stop=(k == KT - 1 and not use_bqk))
                    if use_bqk:
                        bqkt = p2.tile([1, 128], DT_H, tag="bqk_k", name="bqk_k")
                        nc.sync.dma_start(
                            bqkt[:],
                            bqk_d.ap()[l][0:1, QK_PAD + m * 128:QK_PAD + (m + 1) * 128])
                        nc.tensor.matmul(ps[:], bqkt[:], ones_row[0:1, 0:KC],
                                         start=False, stop=True)
                    kst = p2.tile([128, KC], DT_A, tag="k_stage", name="k_stage")
                    nc.vector.tensor_copy(kst[:], ps[:])
                    nc.sync.dma_start(kv_in_k[m * 128:(m + 1) * 128, :], kst[:])

                bvt = p2.tile([1, VW], DT_H, tag="bv_s", name="bv_s")
                nc.sync.dma_start(bvt[:], bv_d.ap()[l])
                psv = [[psmm.tile([128, nlen], F32, tag="mm", name="mm")
                        for (n0, nlen) in ((0, 512), (512, VW - 512))]
                       for _ in range(2)]
                for k in range(KT):
                    wvt = p2.tile([128, VW], DT_H, tag="wv_s", name="wv_s")
                    nc.sync.dma_start(wvt[:],
                                      wv_d.ap()[l][k * 128:(k + 1) * 128, :])
                    for ti in range(2):
                        t = 2 * chunk + ti
                        for ni, (n0, nlen) in enumerate(((0, 512), (512, VW - 512))):
                            nc.tensor.matmul(psv[ti][ni][:],
                                             hT_l[k][:, t * 128:(t + 1) * 128],
                                             wvt[:, n0:n0 + nlen],
                                             start=(k == 0), stop=False)
                for ti in range(2):
                    vst = p2.tile([128, VW], DT_A, tag="v_stage", name="v_stage")
                    for ni, (n0, nlen) in enumerate(((0, 512), (512, VW - 512))):
                        nc.tensor.matmul(psv[ti][ni][:], ones_h[:],
                                         bvt[0:1, n0:n0 + nlen], start=False, stop=True)
                        nc.vector.tensor_copy(vst[:, n0:n0 + nlen], psv[ti][ni][:])
                    nc.sync.dma_start(kv_in_v[ti * 128:(ti + 1) * 128, :], vst[:])

                if os.environ.get("FAKE_AG"):
                    for c in range(2):
                        nc.sync.dma_start(kv_out[c, :], kv_in[:])
                else:
                    nc.gpsimd.collective_compute(
                        "AllGather", OP.bypass,
                        replica_groups=[[0, 1], [2, 3], [4, 5], [6, 7]],
                        ins=[kv_in[:].opt()],
                        outs=[kv_out[:].opt()],
                    )
                return kv_out

            def assemble_chunk(kv_out, chunk, kT_full, v_full):
                for c in range(2):
                    kv_k_c = kv_out[c, 0:KELEMC].rearrange("(r t) -> r t", t=KC)
                    for h in range(H):
                        ft, qb = h // 2, (h % 2) * 64
                        nc.sync.dma_start(
                            kT_full[ft][qb:qb + 48,
                                        c * T + chunk * KC:c * T + chunk * KC + KC],
                            kv_k_c[48 * h:48 * h + 48, :])
                    for jj in range(2):
                        kt = c * 4 + chunk * 2 + jj
                        nc.sync.dma_start(
                            v_full[kt][:],
                            kv_out[c, KELEMC + jj * 128 * VW:
                                   KELEMC + (jj + 1) * 128 * VW]
                            .rearrange("(p f) -> p f", p=128))

            # ---- prologue: LN1 + K/V + AllGather for layer 0 ----
            hT = [p1.tile([128, T], DT_H, tag=f"hT{k}", name=f"hT{k}")
                  for k in range(KT)]
            layernorm_to(x, hT, range(TT))
            kv_cur = [kv_pipeline(0, hT, 0), kv_pipeline(0, hT, 1)]

            for l in range(l_run):
                last = (l == L - 1)
                NQT = 1 if last else TT          # query token tiles this layer
                NQ = NQT * 128

                # ================= attention =================
                if use_bqk:
                    bqkt = p2.tile([1, QK_PAD], DT_H, tag="bqk_s", name="bqk_s")
                    nc.sync.dma_start(bqkt[:], bqk_d.ap()[l][0:1, 0:QK_PAD])

                # Q projection (padded feature-major), stays local
                qT = []
                for mp in range(4):
                    wt = p2.tile([128, KT, 256], DT_H, tag="wq_s", name="wq_s")
                    nc.sync.dma_start(
                        wt[:], wq_d.ap()[l][:, mp * 256:(mp + 1) * 256]
                        .rearrange("(o p) n -> p o n", p=128))
                    for mm_ in range(2):
                        m = 2 * mp + mm_
                        ps = psmm.tile([128, NQ], F32, tag="mm", name="mm")
                        for k in range(KT):
                            nc.tensor.matmul(ps[:], wt[:, k, mm_ * 128:(mm_ + 1) * 128],
                                             hT[k][:, 0:NQ], start=(k == 0),
                                             stop=(k == KT - 1 and not use_bqk))
                        if use_bqk:
                            nc.tensor.matmul(ps[:], bqkt[0:1, m * 128:(m + 1) * 128],
                                             ones_row[0:1, 0:NQ], start=False, stop=True)
                        qt = p1.tile([128, NQ], DT_A, tag=f"qT{m}", name=f"qT{m}")
                        nc.vector.tensor_copy(qt[:], ps[:])
                        qT.append(qt)

                # assemble full-sequence K (dense 48-row heads inside padded
                # 64-row slots) and V (token-major)
                kT_full = [p1.tile([128, S], DT_A, tag=f"kTf{r}", name=f"kTf{r}") for r in range(8)]
                v_full = [p1.tile([128, VW], DT_A, tag=f"vf{j}", name=f"vf{j}") for j in range(8)]
                assemble_chunk(kv_cur[0], 0, kT_full, v_full)
                assemble_chunk(kv_cur[1], 1, kT_full, v_full)

                # ---- attention: scores - dist -> exp(slope*.) -> AV [q,vcol] ----
                # kt order puts chunk-0 key blocks first so compute starts as
                # soon as the first AllGather lands.
                KT_ORDER = (0, 1, 4, 5, 2, 3, 6, 7)
                o_sb = [p1.tile([128, D], DT_A, tag=f"osb{qt}", name=f"osb{qt}")
                        for qt in range(NQT)]
                for hh in range(H):
                    j, qb = hh // 2, (hh % 2) * 64
                    psav = [psmm.tile([128, 49], F32, tag="mm", name="mm")
                            for _ in range(NQT)]
                    for ki, kt in enumerate(KT_ORDER):
                        # bias subtract: mostly on the PE (extra -I*dist
                        # accumulation matmul; slope lives in the exp scale),
                        # a slice on the DVE to balance engine load.
                        on_dve = (8 * hh + kt) % 5 == 0
                        pss = psmm.tile([128, NQ], F32, tag="mm", name="mm")
                        nc.tensor.matmul(
                            pss[:],
                            kT_full[j][qb:qb + 48, kt * 128:(kt + 1) * 128],
                            qT[j][qb:qb + 48, 0:NQ], start=True, stop=on_dve)
                        pt = p4.tile([128, NQ], DT_A, tag="p", name="p")
                        if on_dve:
                            ut = p4.tile([128, NQ], DT_A, tag="u", name="u")
                            nc.vector.tensor_tensor(ut[:], pss[:],
                                                    distT[kt][:, 0:NQ], OP.subtract)
                            nc.scalar.activation(pt[:], ut[:], AF.Exp,
                                                 scale=float(slopes[hh]))
                        else:
                            nc.tensor.matmul(pss[:], negid[:], distT[kt][:, 0:NQ],
                                             start=False, stop=True)
                            nc.scalar.activation(pt[:], pss[:], AF.Exp,
                                                 scale=float(slopes[hh]))
                        for qt in range(NQT):
                            nc.tensor.matmul(
                                psav[qt][:],
                                pt[:, qt * 128:(qt + 1) * 128],
                                v_full[kt][:, 49 * hh:49 * hh + 49],
                                start=(ki == 0), stop=(ki == 7))
                    for qt in range(NQT):
                        rec = p4.tile([128, 1], F32, tag="rec", name="rec")
                        nc.vector.reciprocal(rec[:], psav[qt][:, 48:49])
                        nc.vector.tensor_scalar(
                            o_sb[qt][:, 48 * hh:48 * hh + 48],
                            psav[qt][:, 0:48],
                            rec[:], None, OP.mult)

                # transpose o to feature-major, then dense Wo + residual
                oT = [p2.tile([128, NQ], DT_A, tag=f"oT{d}", name=f"oT{d}")
                      for d in range(KT)]
                for qt in range(NQT):
                    for d in range(KT):
                        pst = psmm.tile([128, 128], DT_A, tag="mm", name="mm")
                        nc.tensor.transpose(pst[:], o_sb[qt][:, d * 128:(d + 1) * 128],
                                            ident_a[:])
                        nc.vector.tensor_copy(oT[d][:, qt * 128:(qt + 1) * 128], pst[:])

                psw = [[psmm.tile([128, nlen], F32, tag="mm", name="mm")
                        for (n0, nlen) in ((0, 512), (512, 256))]
                       for t in range(NQT)]
                for k in range(KT):
                    wot = p3.tile([128, D], DT_A, tag="wo_s", name="wo_s")
                    nc.sync.dma_start(wot[:],
                                      wo_d.ap()[l][k * 128:(k + 1) * 128, :])
                    for t in range(NQT):
                        for ni, (n0, nlen) in enumerate(((0, 512), (512, 256))):
                            nc.tensor.matmul(psw[t][ni][:],
                                             oT[k][:, t * 128:(t + 1) * 128],
                                             wot[:, n0:n0 + nlen], start=(k == 0),
                                             stop=(k == KT - 1))
                for t in range(NQT):
                    for ni, (n0, nlen) in enumerate(((0, 512), (512, 256))):
                        nc.vector.tensor_tensor(x[t][:, n0:n0 + nlen],
                                                x[t][:, n0:n0 + nlen],
                                                psw[t][ni][:], OP.add)

                # ================= SwiGLU FFN =================
                h2T = [p1.tile([128, NQ], DT_H, tag=f"h2T{k}", name=f"h2T{k}") for k in range(KT)]
                layernorm_to(x, h2T, range(NQT))

                if use_bgu:
                    bgt = p2.tile([1, FF], DT_H, tag="bg_s", name="bg_s")
                    nc.sync.dma_start(bgt[:], bg_d.ap()[l])
                    but = p2.tile([1, FF], DT_H, tag="bu_s", name="bu_s")
                    nc.sync.dma_start(but[:], bu_d.ap()[l])
                h3 = []
                for f in range(FT):
                    wgu = p2.tile([128, KT, 256], DT_H, tag="wgu_s", name="wgu_s")
                    nc.sync.dma_start(
                        wgu[:], wgu_d.ap()[l][:, f * 256:(f + 1) * 256]
                        .rearrange("(o p) n -> p o n", p=128))
                    psg = psmm.tile([128, NQ], F32, tag="mm", name="mm")
                    for k in range(KT):
                        nc.tensor.matmul(psg[:], wgu[:, k, 0:128], h2T[k][:],
                                         start=(k == 0),
                                         stop=(k == KT - 1 and not use_bgu))
                    if use_bgu:
                        nc.tensor.matmul(psg[:], bgt[0:1, f * 128:(f + 1) * 128],
                                         ones_row[0:1, 0:NQ], start=False, stop=True)
                    gsb = p2.tile([128, NQ], DT_A, tag="g_sb", name="g_sb")
                    nc.scalar.activation(gsb[:], psg[:], AF.Silu)
                    psu = psmm.tile([128, NQ], F32, tag="mm", name="mm")
                    for k in range(KT):
                        nc.tensor.matmul(psu[:], wgu[:, k, 128:256], h2T[k][:],
                                         start=(k == 0),
                                         stop=(k == KT - 1 and not use_bgu))
                    if use_bgu:
                        nc.tensor.matmul(psu[:], but[0:1, f * 128:(f + 1) * 128],
                                         ones_row[0:1, 0:NQ], start=False, stop=True)
                    h3f = p1.tile([128, NQ], DT_A, tag=f"h3_{f}", name=f"h3_{f}")
                    nc.vector.tensor_tensor(h3f[:], gsb[:], psu[:], OP.mult)
                    h3.append(h3f)

                psd = [[psmm.tile([128, nlen], F32, tag="mm", name="mm")
                        for (n0, nlen) in ((0, 512), (512, 256))] for t in range(NQT)]
                for f in range(FT):
                    wdt = p3.tile([128, D], DT_A, tag="wd_s", name="wd_s")
                    nc.sync.dma_start(wdt[:], wd_d.ap()[l][f * 128:(f + 1) * 128, :])
                    for t in range(NQT):
                        for ni, (n0, nlen) in enumerate(((0, 512), (512, 256))):
                            nc.tensor.matmul(psd[t][ni][:],
                                             h3[f][:, t * 128:(t + 1) * 128],
                                             wdt[:, n0:n0 + nlen],
                                             start=(f == 0), stop=(f == FT - 1))
                if l + 1 < l_run:
                    # residual adds + next layer's LN1 + K/V + AllGather,
                    # interleaved so the collectives overlap the FFN tail
                    # and next layer's attention prologue.
                    hT_next = [p1.tile([128, T], DT_H, tag=f"hT{k}", name=f"hT{k}")
                               for k in range(KT)]
                    for t in (0, 1):
                        for ni, (n0, nlen) in enumerate(((0, 512), (512, 256))):
                            nc.vector.tensor_tensor(x[t][:, n0:n0 + nlen],
                                                    x[t][:, n0:n0 + nlen],
                                                    psd[t][ni][:], OP.add)
                    layernorm_to(x, hT_next, (0, 1))
                    for t in (2, 3):
                        for ni, (n0, nlen) in enumerate(((0, 512), (512, 256))):
                            nc.vector.tensor_tensor(x[t][:, n0:n0 + nlen],
                                                    x[t][:, n0:n0 + nlen],
                                                    psd[t][ni][:], OP.add)
                    kv_a = kv_pipeline(l + 1, hT_next, 0)
                    layernorm_to(x, hT_next, (2, 3))
                    kv_b = kv_pipeline(l + 1, hT_next, 1)
                    hT = hT_next
                    kv_cur = [kv_a, kv_b]
                else:
                    for t in range(NQT):
                        for ni, (n0, nlen) in enumerate(((0, 512), (512, 256))):
                            nc.vector.tensor_tensor(x[t][:, n0:n0 + nlen],
                                                    x[t][:, n0:n0 + nlen],
                                                    psd[t][ni][:], OP.add)

            # ---- final layernorm of the CLS row (token 0) + affine ----
            finw = p1.tile([1, D], F32, tag="finw", name="finw")
            nc.sync.dma_start(finw[:], finw_d.ap())
            finb = p1.tile([1, D], F32, tag="finb", name="finb")
            nc.sync.dma_start(finb[:], finb_d.ap())

            x0r = x[0][0:1, :]
            fst = p2.tile([1, 12], F32, tag="fbnst", name="fbnst")
            nc.vector.bn_stats(fst[:, 0:6], x0r[:, 0:384])
            nc.vector.bn_stats(fst[:, 6:12], x0r[:, 384:768])
            fag = p2.tile([1, 2], F32, tag="fbnag", name="fbnag")
            nc.vector.bn_aggr(fag[:], fst[:])
            nmean = p2.tile([1, 1], F32, tag="fnmean", name="fnmean")
            nc.scalar.mul(nmean[:], fag[:, 0:1], -1.0)
            xc = p1.tile([1, D], F32, tag="fxc", name="fxc")
            nc.vector.tensor_scalar(xc[:], x0r, nmean[:], None, OP.add)
            stdt = p2.tile([1, 1], F32, tag="fstd", name="fstd")
            nc.scalar.activation(stdt[:], fag[:, 1:2], AF.Sqrt, bias=epst[0:1, :])
            rstd = p2.tile([1, 1], F32, tag="frstd", name="frstd")
            nc.vector.reciprocal(rstd[:], stdt[:])
            yt = p1.tile([1, D], F32, tag="fy", name="fy")
            nc.vector.tensor_scalar(yt[:], xc[:], rstd[:], None, OP.mult)
            nc.vector.tensor_tensor(yt[:], yt[:], finw[:], OP.mult)
            nc.vector.tensor_tensor(yt[:], yt[:], finb[:], OP.add)
            nc.sync.dma_start(y_d.ap(), yt[:])

    nc.compile()
    return nc


def prepare_inputs(cls_tokens, cls_token, log_slopes, Wqkv, Wo, Wg, Wu, Wd,
                   ln1_w, ln1_b, ln2_w, ln2_b, fin_w, fin_b):
    """Fold LN affine params into weights, pad Q heads, build per-core arrays."""
    f32 = np.float32
    scale = 1.0 / math.sqrt(HD)

    slopes_np = np.exp(np.asarray(log_slopes, np.float64))
    wq = np.zeros((L, D, QK_PAD), f32)
    wk = np.zeros((L, D, D), f32)
    bqk = np.zeros((L, 1, QK_PAD + D), f32)
    wv = np.zeros((L, D, VW), f32)
    bv = np.zeros((L, 1, VW), f32)
    wo = np.zeros((L, D, D), f32)
    wgu = np.zeros((L, D, 2 * FF), f32)
    bg = np.zeros((L, 1, FF), f32)
    bu = np.zeros((L, 1, FF), f32)
    wd = np.zeros((L, FF, D), f32)

    for l in range(L):
        W1 = (np.asarray(Wqkv[l], np.float64) *
              np.asarray(ln1_w[l], np.float64)[None, :])
        b1 = np.asarray(Wqkv[l], np.float64) @ np.asarray(ln1_b[l], np.float64)
        for h in range(H):
            qs = slice(48 * h, 48 * h + 48)
            qscale = scale / slopes_np[h]
            wq[l, :, 64 * h:64 * h + 48] = (W1[qs].T * qscale)
            bqk[l, 0, 64 * h:64 * h + 48] = b1[qs] * qscale
            ks = slice(D + 48 * h, D + 48 * h + 48)
            wk[l, :, 48 * h:48 * h + 48] = W1[ks].T
            bqk[l, 0, QK_PAD + 48 * h:QK_PAD + 48 * h + 48] = b1[ks]
            vs = slice(2 * D + 48 * h, 2 * D + 48 * h + 48)
            wv[l, :, 49 * h:49 * h + 48] = W1[vs].T
            bv[l, 0, 49 * h:49 * h + 48] = b1[vs]
            bv[l, 0, 49 * h + 48] = 1.0
        wo[l] = np.asarray(Wo[l], f32).T
        W2g = (np.asarray(Wg[l], np.float64) *
               np.asarray(ln2_w[l], np.float64)[None, :])
        W2u = (np.asarray(Wu[l], np.float64) *
               np.asarray(ln2_w[l], np.float64)[None, :])
        wgu_l = wgu[l].reshape(D, FT, 2, 128)
        wgu_l[:, :, 0, :] = W2g.T.reshape(D, FT, 128)
        wgu_l[:, :, 1, :] = W2u.T.reshape(D, FT, 128)
        bg[l, 0] = np.asarray(Wg[l], np.float64) @ np.asarray(ln2_b[l], np.float64)
        bu[l, 0] = np.asarray(Wu[l], np.float64) @ np.asarray(ln2_b[l], np.float64)
        wd[l] = np.asarray(Wd[l], f32).T

    use_bqk = bool(np.any(bqk != 0))
    use_bgu = bool(np.any(bg != 0) or np.any(bu != 0))

    x_full = np.concatenate(
        [np.broadcast_to(np.asarray(cls_token, f32), (B, 1, D)),
         np.asarray(cls_tokens, f32)], axis=1)  # (B, S, D)

    np_a = mybir.dt.np(DT_A)
    np_h = mybir.dt.np(DT_H)
    common = {
        "wq": wq.astype(np_h), "wk": wk.astype(np_h),
        "wv": wv.astype(np_h), "bv": bv.astype(np_h),
        "wo": wo.astype(np_a), "wgu": wgu.astype(np_h),
        "wd": wd.astype(np_a),
        "finw": np.asarray(fin_w, f32).reshape(1, D),
        "finb": np.asarray(fin_b, f32).reshape(1, D),
    }
    if use_bqk:
        common["bqk"] = bqk.astype(np_h)
    if use_bgu:
        common["bg"] = bg.astype(np_h)
        common["bu"] = bu.astype(np_h)

    ks = np.arange(S, dtype=np.float64)
    in_maps = []
    for c in range(NCORES):
        b, half = c // 2, c % 2
        q0 = T * half
        dist = np.abs((q0 + np.arange(T, dtype=np.float64))[None, :] - ks[:, None])
        m = dict(common)
        m["x0"] = np.ascontiguousarray(x_full[b, q0:q0 + T])
        m["dist"] = dist.astype(np.float16)
        in_maps.append(m)
    return in_maps, use_bqk, use_bgu


def kernel(**inputs):
    in_maps, use_bqk, use_bgu = prepare_inputs(**inputs)
    slopes = np.exp(np.asarray(inputs["log_slopes"], np.float64))
    key = (use_bqk, use_bgu, tuple(np.round(slopes, 10)))
    if key not in _NC_CACHE:
        _NC_CACHE[key] = build_nc(use_bqk, use_bgu, slopes=slopes)
    nc = _NC_CACHE[key]
    res = run_bass_kernel_spmd(nc, in_maps, core_ids=list(range(NCORES)))
    out = np.stack([res.results[2 * b]["y"][0] for b in range(B)])
    return out.astype(np.float32)
